# revision 7
# baseline (speedup 1.0000x reference)
"""Sliding-window (radius-8, K=17) single-head attention along W.

Full problem: feature/position [2, 128, 64, 256] f32; 1x1 convs Wq/Wk (+bias)
produce q/k; scores over a 17-wide window along W; softmax (zero-padded
windows contribute exp(0)=1 to the denominator); output is the attn-weighted
sum of windows of x = feature + position.

Sharding: data-parallel over (B, H) — the 128 (b, h) rows are independent;
each of the 8 cores gets 16 rows, two per iteration.

Per row (x_row = [C=128, W=256]):
  q = (Wq/sqrt(C)) x + bq/sqrt(C);  k = Wk x + bk        (PE matmuls + bias)
  S^T[w', w] = k^T q   computed TRANSPOSED (keys on partitions) so exp(S^T)
      lands in SBUF in the layout the den/out matmuls need.
  Band structure: key chunk 1 (keys 0..127) only reaches queries 0..135;
  chunk 2 (keys 128..255) only queries 120..255. exp / mask / den / out all
  operate on those 136-wide strips only (scores are computed full-width —
  f32r matmuls need >=256 moving cols for 1 cyc/row — but never read
  outside the strips). Masking is multiplicative 0/1 on exp(S) post-exp.
  den[w] (broadcast across partitions) = ones128.T @ att strips, PSUM-
      initialized by ones128.T @ (oobcount/128) for the zero-padded
      out-of-range taps (exp(0)=1 each); out = (x^T.T @ att) * recip(den).
  x^T chunks from PE transposes of the f32r x (f32r identity moving).

Schedule: two-stage software pipeline. Stage A(i) = x-add (4-row blocks on
gpsimd), q/k matmuls + bias evictions, score matmuls, exp, mask, x^T
transposes + eviction. Stage B(i) = den matmuls, reciprocal, out matmuls,
final normalize, output DMA. Issue order A(0), A(1), B(0), A(2), B(1), ...
so every engine queue always holds ready work from the adjacent iteration
while stage B of the previous one waits on cross-engine results.

Precision: score path f32r end-to-end; value path (att, x^T, out matmuls)
bf16; scores accumulate in fp32 PSUM; softmax skips max-subtraction.
"""

import numpy as np
from contextlib import ExitStack

import concourse.bacc as bacc
import concourse.mybir as mybir
import concourse.tile as tile
from concourse.ap import AP
from concourse.bass_utils import run_bass_kernel_spmd

B, C, H, W = 2, 128, 64, 256
R = 8
NCORES = 8
ROWS = B * H // NCORES        # 16 (b, h) rows per core
CORES_PER_B = NCORES // B     # 4
F32 = mybir.dt.float32
F32R = mybir.dt.float32r
BF = mybir.dt.bfloat16
EXP = mybir.ActivationFunctionType.Exp
COPY = mybir.ActivationFunctionType.Copy
RL = 4                        # rows per input DMA block / x-add block
SW = 136                      # strip width: chunk1 queries [0:136), chunk2 [120:256)
A1 = 2 * W - SW               # chunk2 strip start within a row's 512 att cols (376)


def strip2(tile_ap, row_off, pstride):
    """[C, 2, SW] view of the two valid strips of one row's 512 cols."""
    v = tile_ap
    return AP(v.tensor, v.offset + row_off, [(pstride, C), (A1, 2), (1, SW)])


def strip4(tile_ap, pstride):
    """[C, 2, 2, SW] view of both rows' strips of a [C, 2, 512] tile."""
    v = tile_ap
    return AP(v.tensor, v.offset, [(pstride, C), (2 * W, 2), (A1, 2), (1, SW)])


def build_nc():
    nc = bacc.Bacc(trn_type="TRN2")
    f_ext = nc.dram_tensor("feature", [C, ROWS, W], F32, kind="ExternalInput")
    p_ext = nc.dram_tensor("position", [C, ROWS, W], F32, kind="ExternalInput")
    wq_ext = nc.dram_tensor("wqt", [C, C], F32R, kind="ExternalInput")
    wk_ext = nc.dram_tensor("wkt", [C, C], F32R, kind="ExternalInput")
    id_ext = nc.dram_tensor("ident", [C, C], F32R, kind="ExternalInput")
    ones_ext = nc.dram_tensor("ones", [C, C], BF, kind="ExternalInput")
    bq_ext = nc.dram_tensor("bqv", [C, 1], F32, kind="ExternalInput")
    bk_ext = nc.dram_tensor("bkv", [C, 1], F32, kind="ExternalInput")
    mask_ext = nc.dram_tensor("maskc", [C, 2, 2, SW], BF, kind="ExternalInput")
    oob_ext = nc.dram_tensor("oob_bc", [C, 2 * W], BF, kind="ExternalInput")
    out_ext = nc.dram_tensor("out", [C, ROWS, W], F32, kind="ExternalOutput")

    with tile.TileContext(nc) as tc, ExitStack() as ctx:
        const = ctx.enter_context(tc.tile_pool(name="const", bufs=1))
        inp = ctx.enter_context(tc.tile_pool(name="inp", bufs=3))

        # first input block loads before the constants so compute starts early
        fts, pts = {}, {}

        def load_block(k):
            ft = inp.tile([C, RL, W], F32, tag="ft")
            nc.sync.dma_start(ft[:], f_ext[:, k * RL : (k + 1) * RL, :])
            pt = inp.tile([C, RL, W], F32, tag="pt")
            nc.sync.dma_start(pt[:], p_ext[:, k * RL : (k + 1) * RL, :])
            fts[k], pts[k] = ft, pt

        load_block(0)

        def cload(shape, dt, ext, tag):
            t = const.tile(shape, dt, tag=tag)
            nc.sync.dma_start(t[:], ext[:])
            return t

        wq_t = cload([C, C], F32R, wq_ext, "wq")
        wk_t = cload([C, C], F32R, wk_ext, "wk")
        ident = cload([C, C], F32R, id_ext, "id")
        ones_t = cload([C, C], BF, ones_ext, "ones")
        bq_t = cload([C, 1], F32, bq_ext, "bq")
        bk_t = cload([C, 1], F32, bk_ext, "bk")
        mask_t = cload([C, 2, 2, SW], BF, mask_ext, "mask")
        oob_t = cload([C, 2 * W], BF, oob_ext, "oob")

        load_block(1)

        # touch Exp once so the ACT table loads during the input-DMA ramp
        warm = const.tile([C, 1], F32, tag="warm")
        nc.scalar.activation(warm[:], bq_t[:], EXP)

        xp = ctx.enter_context(tc.tile_pool(name="x", bufs=2))
        qkp = ctx.enter_context(tc.tile_pool(name="qk", bufs=2))
        attp = ctx.enter_context(tc.tile_pool(name="att", bufs=2))
        sbT = ctx.enter_context(tc.tile_pool(name="sbT", bufs=2))
        rdp = ctx.enter_context(tc.tile_pool(name="rd", bufs=2))
        osp = ctx.enter_context(tc.tile_pool(name="os", bufs=2))
        psq = ctx.enter_context(tc.tile_pool(name="psq", bufs=1, space="PSUM"))
        psk = ctx.enter_context(tc.tile_pool(name="psk", bufs=1, space="PSUM"))
        pss = ctx.enter_context(tc.tile_pool(name="pss", bufs=2, space="PSUM"))
        psden = ctx.enter_context(tc.tile_pool(name="psden", bufs=1, space="PSUM"))
        psxt = ctx.enter_context(tc.tile_pool(name="psxt", bufs=1, space="PSUM"))
        pso = ctx.enter_context(tc.tile_pool(name="pso", bufs=2, space="PSUM"))

        NIT = ROWS // 2
        xs = {}
        st = {}

        def stageA(it):
            r = 2 * it
            blk = r // RL
            j = r % RL
            if j == 0:
                if blk + 2 < ROWS // RL:
                    load_block(blk + 2)
                # x for the whole block in one gpsimd add: [C, RL, W] f32r
                x4 = xp.tile([C, RL, W], F32R, tag="x4")
                nc.gpsimd.tensor_add(x4[:], fts[blk][:], pts[blk][:])
                xs[blk] = x4
            x4 = xs[blk]

            # q and k for both rows in one matmul each (512 moving cols)
            q_ps = psq.tile([C, 2 * W], F32, tag="q")
            nc.tensor.matmul(q_ps[:], wq_t[:], x4[:, j : j + 2, :], start=True, stop=True)
            k_ps = psk.tile([C, 2 * W], F32, tag="k")
            nc.tensor.matmul(k_ps[:], wk_t[:], x4[:, j : j + 2, :], start=True, stop=True)
            q_sb = qkp.tile([C, 2 * W], F32R, tag="q")
            nc.scalar.add(q_sb[:], q_ps[:], bq_t[:])
            k_sb = qkp.tile([C, 2 * W], F32R, tag="k")
            nc.vector.tensor_scalar_add(k_sb[:], k_ps[:], bk_t[:])

            # scores per row, transposed; exp on the valid strips only
            att = attp.tile([C, 2, 2 * W], BF)
            att_pstride = 2 * 2 * W
            for rr in range(2):
                q0 = rr * W
                s_ps = pss.tile([C, 2 * W], F32, tag="s")
                nc.tensor.matmul(
                    s_ps[:, 0:W],
                    k_sb[:, q0 : q0 + 128],
                    q_sb[:, q0 : q0 + W],
                    start=True, stop=True,
                )
                nc.tensor.matmul(
                    s_ps[:, W : 2 * W],
                    k_sb[:, q0 + 128 : q0 + W],
                    q_sb[:, q0 : q0 + W],
                    start=True, stop=True,
                )
                nc.scalar.activation(
                    strip2(att[:], rr * 2 * W, att_pstride),
                    strip2(s_ps[:], 0, 2 * W),
                    EXP,
                )
            # multiplicative 0/1 band mask, both rows in one op
            av = strip4(att[:], att_pstride)
            nc.vector.tensor_mul(av, av, mask_t[:])

            # x^T chunks for the output matmul: f32r transposes, bf16 evict
            xt_ps = psxt.tile([C, 2 * W], F32R, tag="xt")
            for rr in range(2):
                nc.tensor.transpose(
                    xt_ps[:, rr * W : rr * W + 128], x4[:, j + rr, 0:128], ident[:]
                )
                nc.tensor.transpose(
                    xt_ps[:, rr * W + 128 : (rr + 1) * W], x4[:, j + rr, 128:256], ident[:]
                )
            xT = sbT.tile([C, 2 * W], BF, tag="xT")
            nc.scalar.activation(xT[:, 0:W], xt_ps[:, 0:W], COPY)
            nc.vector.tensor_copy(xT[:, W : 2 * W], xt_ps[:, W : 2 * W])
            st[it] = (att, xT)

        def stageB(it):
            r = 2 * it
            att, xT = st.pop(it)
            # denominators, broadcast across partitions by the ones matmul;
            # PSUM-initialized with the oob counts (pre-divided by 128).
            den_ps = psden.tile([C, 2 * W], F32, tag="den")
            nc.tensor.matmul(den_ps[:], ones_t[:], oob_t[:], start=True, stop=False)
            for rr in range(2):
                d0 = rr * W
                nc.tensor.matmul(
                    den_ps[:, d0 : d0 + SW],
                    ones_t[:],
                    att[:, rr, 0:SW],
                    start=False, stop=False,
                )
                nc.tensor.matmul(
                    den_ps[:, d0 + W - SW : d0 + W],
                    ones_t[:],
                    att[:, rr, A1 : 2 * W],
                    start=False, stop=(rr == 1),
                )
            rden = rdp.tile([C, 2 * W], F32)
            nc.vector.reciprocal_approx_fast(out=rden[:], in_=den_ps[:])

            o_ps = pso.tile([C, 2 * W], F32, tag="out")
            for rr in range(2):
                o0 = rr * W
                nc.tensor.matmul(
                    o_ps[:, o0 : o0 + SW],
                    xT[:, o0 : o0 + 128],
                    att[:, rr, 0:SW],
                    start=True, stop=False,
                )
                nc.tensor.matmul(
                    o_ps[:, o0 + W - SW : o0 + SW],
                    xT[:, o0 + 128 : o0 + W],
                    att[:, rr, A1 : A1 + 16],
                    start=False, stop=True,
                )
                nc.tensor.matmul(
                    o_ps[:, o0 + SW : o0 + W],
                    xT[:, o0 + 128 : o0 + W],
                    att[:, rr, A1 + 16 : 2 * W],
                    start=True, stop=True,
                )
            o_sb = osp.tile([C, 2 * W], F32, tag="osb")
            nc.vector.tensor_mul(o_sb[:], o_ps[:], rden[:])
            nc.sync.dma_start(out_ext[:, r : r + 2, :], o_sb[:])

        stageA(0)
        for it in range(1, NIT):
            stageA(it)
            stageB(it - 1)
        stageB(NIT - 1)

    nc.compile()
    return nc


def host_consts(Wq, bq, Wk, bk):
    import ml_dtypes

    sc = 1.0 / np.sqrt(np.float32(C))
    wqt = np.ascontiguousarray(Wq.astype(np.float32).T * sc)
    bqv = np.ascontiguousarray((bq.astype(np.float32) * sc).reshape(C, 1))
    wkt = np.ascontiguousarray(Wk.astype(np.float32).T)
    bkv = np.ascontiguousarray(bk.astype(np.float32).reshape(C, 1))

    ident = np.eye(C, dtype=np.float32)
    ones = np.ones((C, C), dtype=np.float32).astype(ml_dtypes.bfloat16)

    # 0/1 band masks on the two valid strips (same for both rows):
    # chunk1: key p vs query w in [0, SW);  chunk2: key 128+p vs query 120+j
    maskc = np.zeros((C, 2, SW), dtype=np.float32)
    for p in range(C):
        for w in range(SW):
            if abs(p - w) <= R:
                maskc[p, 0, w] = 1.0
            if abs((128 + p) - (W - SW + w)) <= R:
                maskc[p, 1, w] = 1.0
    maskc = np.broadcast_to(maskc[:, None], (C, 2, 2, SW))
    maskc = np.ascontiguousarray(maskc).astype(ml_dtypes.bfloat16)

    # oob count per query w (pre-divided by 128: the ones-matmul sums over
    # 128 partitions), same row repeated on all partitions, two rows
    wgrid = np.arange(W)
    oob_row = (np.maximum(0, R - wgrid) + np.maximum(0, wgrid - (W - 1 - R))) / 128.0
    oob_bc = np.tile(oob_row.astype(np.float32), (C, 2)).astype(ml_dtypes.bfloat16)
    return wqt, bqv, wkt, bkv, maskc, oob_bc, ident, ones


def core_inputs(feature, position, Wq, bq, Wk, bk):
    wqt, bqv, wkt, bkv, maskc, oob_bc, ident, ones = host_consts(Wq, bq, Wk, bk)
    in_maps = []
    for i in range(NCORES):
        b = i // CORES_PER_B
        h0 = (i % CORES_PER_B) * ROWS
        in_maps.append(
            {
                "feature": np.ascontiguousarray(
                    feature[b, :, h0 : h0 + ROWS, :], dtype=np.float32
                ),
                "position": np.ascontiguousarray(
                    position[b, :, h0 : h0 + ROWS, :], dtype=np.float32
                ),
                "wqt": wqt,
                "ident": ident,
                "ones": ones,
                "wkt": wkt,
                "bqv": bqv,
                "bkv": bkv,
                "maskc": maskc,
                "oob_bc": oob_bc,
            }
        )
    return in_maps


def kernel(feature, position, Wq, bq, Wk, bk):
    feature = np.asarray(feature, dtype=np.float32)
    position = np.asarray(position, dtype=np.float32)
    Wq = np.asarray(Wq, dtype=np.float32)
    bq = np.asarray(bq, dtype=np.float32)
    Wk = np.asarray(Wk, dtype=np.float32)
    bk = np.asarray(bk, dtype=np.float32)
    in_maps = core_inputs(feature, position, Wq, bq, Wk, bk)
    nc = build_nc()
    res = run_bass_kernel_spmd(nc, in_maps, list(range(NCORES)))
    out = np.empty((B, C, H, W), dtype=np.float32)
    for i in range(NCORES):
        b = i // CORES_PER_B
        h0 = (i % CORES_PER_B) * ROWS
        out[b, :, h0 : h0 + ROWS, :] = res.results[i]["out"]
    return out


# revision 9
# speedup vs baseline: 1.0306x; 1.0306x over previous
"""Sliding-window (radius-8, K=17) single-head attention along W.

Full problem: feature/position [2, 128, 64, 256] f32; 1x1 convs Wq/Wk (+bias)
produce q/k; scores over a 17-wide window along W; softmax (zero-padded
windows contribute exp(0)=1 to the denominator); output is the attn-weighted
sum of windows of x = feature + position.

Sharding: data-parallel over (B, H) — the 128 (b, h) rows are independent;
each of the 8 cores gets 16 rows, two per iteration.

Per row (x_row = [C=128, W=256]):
  q = (Wq/sqrt(C)) x + bq/sqrt(C);  k = Wk x + bk        (PE matmuls + bias)
  S^T[w', w] = k^T q   computed TRANSPOSED (keys on partitions) so exp(S^T)
      lands in SBUF in the layout the den/out matmuls need.
  Band structure: key chunk 1 (keys 0..127) only reaches queries 0..135;
  chunk 2 (keys 128..255) only queries 120..255. exp / mask / den / out all
  operate on those 136-wide strips only (scores are computed full-width —
  f32r matmuls need >=256 moving cols for 1 cyc/row — but never read
  outside the strips). Masking is multiplicative 0/1 on exp(S) post-exp.
  den[w] (broadcast across partitions) = ones128.T @ att strips, PSUM-
      initialized by ones128.T @ (oobcount/128) for the zero-padded
      out-of-range taps (exp(0)=1 each); out = (x^T.T @ att) * recip(den).
  x^T chunks from PE transposes of the f32r x (f32r identity moving).

Schedule: two-stage software pipeline. Stage A(i) = x-add (gpsimd), q/k
matmuls, x^T transposes (issued before the score matmuls so the PE has
dependency-free work while the bias evictions run), bias evictions
(q on scalar, k on scalar), score matmuls, one merged strip-exp (scalar),
one merged strip-mask (vector), x^T eviction (scalar). Stage B(i) = den
matmuls (3: oob init + one per key chunk over both rows), reciprocal
(vector), out matmuls (6), final normalize (vector), batched output DMA.
Issue order A(0), A(1), B(0), A(2), B(1), ... so engine queues always hold
ready work from the adjacent iteration. Input DMAs: rows 0-1 and the small
q/k weights first so compute starts as early as possible.

Precision: score path f32r end-to-end; value path (att, x^T, out matmuls)
bf16; scores accumulate in fp32 PSUM; softmax skips max-subtraction.
"""

import numpy as np
from contextlib import ExitStack

import concourse.bacc as bacc
import concourse.mybir as mybir
import concourse.tile as tile
from concourse.ap import AP
from concourse.bass_utils import run_bass_kernel_spmd

B, C, H, W = 2, 128, 64, 256
R = 8
NCORES = 8
ROWS = B * H // NCORES        # 16 (b, h) rows per core
CORES_PER_B = NCORES // B     # 4
F32 = mybir.dt.float32
F32R = mybir.dt.float32r
BF = mybir.dt.bfloat16
EXP = mybir.ActivationFunctionType.Exp
COPY = mybir.ActivationFunctionType.Copy
SW = 136                      # strip width: chunk1 queries [0:136), chunk2 [120:256)
A1 = 2 * W - SW               # chunk2 strip start within a row's 512 att cols (376)


def ap4(t, pstride, d1, d2, d3, off=0):
    v = t[:]
    return AP(v.tensor, v.offset + off, [(pstride, C), d1, d2, d3])


def ap3(t, pstride, d1, d2, off=0):
    v = t[:]
    return AP(v.tensor, v.offset + off, [(pstride, C), d1, d2])


def build_nc():
    nc = bacc.Bacc(trn_type="TRN2")
    f_ext = nc.dram_tensor("feature", [C, ROWS, W], F32, kind="ExternalInput")
    p_ext = nc.dram_tensor("position", [C, ROWS, W], F32, kind="ExternalInput")
    wq_ext = nc.dram_tensor("wqt", [C, C], F32R, kind="ExternalInput")
    wk_ext = nc.dram_tensor("wkt", [C, C], F32R, kind="ExternalInput")
    id_ext = nc.dram_tensor("ident", [C, C], F32R, kind="ExternalInput")
    ones_ext = nc.dram_tensor("ones", [C, C], BF, kind="ExternalInput")
    bq_ext = nc.dram_tensor("bqv", [C, 1], F32, kind="ExternalInput")
    bk_ext = nc.dram_tensor("bkv", [C, 1], F32, kind="ExternalInput")
    mask_ext = nc.dram_tensor("maskc", [C, 2, 2, SW], BF, kind="ExternalInput")
    oob_ext = nc.dram_tensor("oob_bc", [C, 2 * W], BF, kind="ExternalInput")
    out_ext = nc.dram_tensor("out", [C, ROWS, W], F32, kind="ExternalOutput")

    with tile.TileContext(nc) as tc, ExitStack() as ctx:
        const = ctx.enter_context(tc.tile_pool(name="const", bufs=1))
        inp = ctx.enter_context(tc.tile_pool(name="inp", bufs=3))

        blocks = {}   # iter -> (ft, pt, j): input tile pair + row offset

        def load_rows(r0, nrows, iters):
            ft = inp.tile([C, nrows, W], F32, tag="ft")
            nc.sync.dma_start(ft[:], f_ext[:, r0 : r0 + nrows, :])
            pt = inp.tile([C, nrows, W], F32, tag="pt")
            nc.sync.dma_start(pt[:], p_ext[:, r0 : r0 + nrows, :])
            for n, it in enumerate(iters):
                blocks[it] = (ft, pt, 2 * n)

        def cload(shape, dt, ext, tag):
            t = const.tile(shape, dt, tag=tag)
            nc.sync.dma_start(t[:], ext[:])
            return t

        # rows 0-1 and the q/k weights land first so compute starts early
        load_rows(0, 2, [0])
        wq_t = cload([C, C], F32R, wq_ext, "wq")
        wk_t = cload([C, C], F32R, wk_ext, "wk")
        bq_t = cload([C, 1], F32, bq_ext, "bq")
        bk_t = cload([C, 1], F32, bk_ext, "bk")
        load_rows(2, 2, [1])
        ident = cload([C, C], F32R, id_ext, "id")
        mask_t = cload([C, 2, 2, SW], BF, mask_ext, "mask")
        ones_t = cload([C, C], BF, ones_ext, "ones")
        oob_t = cload([C, 2 * W], BF, oob_ext, "oob")
        load_rows(4, 4, [2, 3])

        # touch Exp once so the ACT table loads during the input-DMA ramp
        warm = const.tile([C, 1], F32, tag="warm")
        nc.scalar.activation(warm[:], bq_t[:], EXP)

        xp = ctx.enter_context(tc.tile_pool(name="x", bufs=2))
        qkp = ctx.enter_context(tc.tile_pool(name="qk", bufs=2))
        attp = ctx.enter_context(tc.tile_pool(name="att", bufs=2))
        sbT = ctx.enter_context(tc.tile_pool(name="sbT", bufs=2))
        rdp = ctx.enter_context(tc.tile_pool(name="rd", bufs=2))
        osp = ctx.enter_context(tc.tile_pool(name="os", bufs=2))
        psq = ctx.enter_context(tc.tile_pool(name="psq", bufs=1, space="PSUM"))
        psk = ctx.enter_context(tc.tile_pool(name="psk", bufs=1, space="PSUM"))
        pss = ctx.enter_context(tc.tile_pool(name="pss", bufs=1, space="PSUM"))
        psden = ctx.enter_context(tc.tile_pool(name="psden", bufs=1, space="PSUM"))
        psxt = ctx.enter_context(tc.tile_pool(name="psxt", bufs=1, space="PSUM"))
        pso = ctx.enter_context(tc.tile_pool(name="pso", bufs=2, space="PSUM"))

        NIT = ROWS // 2
        st = {}
        osb = {}

        def stageA(it):
            r = 2 * it
            if it == 2:
                load_rows(8, 4, [4, 5])
            elif it == 4:
                load_rows(12, 4, [6, 7])
            ft, pt, j = blocks[it]

            # x for both rows: [C, 2, W] f32r
            x2 = xp.tile([C, 2, W], F32R, tag="x2")
            nc.gpsimd.tensor_add(x2[:], ft[:, j : j + 2, :], pt[:, j : j + 2, :])

            # q and k for both rows in one matmul each (512 moving cols)
            q_ps = psq.tile([C, 2 * W], F32, tag="q")
            nc.tensor.matmul(q_ps[:], wq_t[:], x2[:], start=True, stop=True)
            k_ps = psk.tile([C, 2 * W], F32, tag="k")
            nc.tensor.matmul(k_ps[:], wk_t[:], x2[:], start=True, stop=True)

            # x^T transposes depend only on x — keep the PE busy while the
            # q/k bias evictions run on scalar
            xt_ps = psxt.tile([C, 2 * W], F32R, tag="xt")
            for rr in range(2):
                nc.tensor.transpose(
                    xt_ps[:, rr * W : rr * W + 128], x2[:, rr, 0:128], ident[:]
                )
                nc.tensor.transpose(
                    xt_ps[:, rr * W + 128 : (rr + 1) * W], x2[:, rr, 128:256], ident[:]
                )

            q_sb = qkp.tile([C, 2 * W], F32R, tag="q")
            nc.scalar.add(q_sb[:], q_ps[:], bq_t[:])
            k_sb = qkp.tile([C, 2 * W], F32R, tag="k")
            nc.scalar.add(k_sb[:], k_ps[:], bk_t[:])

            # scores per row, transposed: [C, 2, 512] across 2 PSUM banks
            s_ps = pss.tile([C, 2, 2 * W], F32, tag="s")
            for rr in range(2):
                q0 = rr * W
                nc.tensor.matmul(
                    s_ps[:, rr, 0:W],
                    k_sb[:, q0 : q0 + 128],
                    q_sb[:, q0 : q0 + W],
                    start=True, stop=True,
                )
                nc.tensor.matmul(
                    s_ps[:, rr, W : 2 * W],
                    k_sb[:, q0 + 128 : q0 + W],
                    q_sb[:, q0 : q0 + W],
                    start=True, stop=True,
                )
            # exp on the valid strips of both rows in one op
            att = attp.tile([C, 2, 2 * W], BF)
            nc.scalar.activation(
                ap4(att, 2 * 2 * W, (2 * W, 2), (A1, 2), (1, SW)),
                ap4(s_ps, 2 * 2 * W, (2 * W, 2), (A1, 2), (1, SW)),
                EXP,
            )
            # multiplicative 0/1 band mask, both rows in one op
            av = ap4(att, 2 * 2 * W, (2 * W, 2), (A1, 2), (1, SW))
            nc.vector.tensor_mul(av, av, mask_t[:])

            xT = sbT.tile([C, 2 * W], BF, tag="xT")
            nc.scalar.activation(xT[:], xt_ps[:], COPY)
            st[it] = (att, xT)

        def stageB(it):
            r = 2 * it
            att, xT = st.pop(it)
            # denominators, broadcast across partitions by the ones matmul;
            # PSUM-initialized with the oob counts (pre-divided by 128).
            den_ps = psden.tile([C, 2 * W], F32, tag="den")
            nc.tensor.matmul(den_ps[:], ones_t[:], oob_t[:], start=True, stop=False)
            nc.tensor.matmul(
                ap3(den_ps, 2 * W, (W, 2), (1, SW)),
                ones_t[:],
                ap3(att, 2 * 2 * W, (2 * W, 2), (1, SW)),
                start=False, stop=False,
            )
            nc.tensor.matmul(
                ap3(den_ps, 2 * W, (W, 2), (1, SW), off=W - SW),
                ones_t[:],
                ap3(att, 2 * 2 * W, (2 * W, 2), (1, SW), off=A1),
                start=False, stop=True,
            )
            rden = rdp.tile([C, 2 * W], F32)
            nc.vector.reciprocal_approx_fast(out=rden[:], in_=den_ps[:])

            o_ps = pso.tile([C, 2 * W], F32, tag="out")
            for rr in range(2):
                o0 = rr * W
                nc.tensor.matmul(
                    o_ps[:, o0 : o0 + SW],
                    xT[:, o0 : o0 + 128],
                    att[:, rr, 0:SW],
                    start=True, stop=False,
                )
                nc.tensor.matmul(
                    o_ps[:, o0 + W - SW : o0 + SW],
                    xT[:, o0 + 128 : o0 + W],
                    att[:, rr, A1 : A1 + 16],
                    start=False, stop=True,
                )
                nc.tensor.matmul(
                    o_ps[:, o0 + SW : o0 + W],
                    xT[:, o0 + 128 : o0 + W],
                    att[:, rr, A1 + 16 : 2 * W],
                    start=True, stop=True,
                )
            # final normalize into a 4-row output buffer; DMA every 2nd iter
            if it % 2 == 0:
                o_sb4 = osp.tile([C, 4, W], F32, tag="osb")
                osb[it // 2] = o_sb4
            o_sb = osb[it // 2]
            half = (it % 2) * 2
            nc.vector.tensor_mul(o_sb[:, half : half + 2, :], o_ps[:], rden[:])
            if it % 2 == 1:
                nc.sync.dma_start(out_ext[:, r - 2 : r + 2, :], o_sb[:])

        stageA(0)
        for it in range(1, NIT):
            stageA(it)
            stageB(it - 1)
        stageB(NIT - 1)

    nc.compile()
    return nc


def host_consts(Wq, bq, Wk, bk):
    import ml_dtypes

    sc = 1.0 / np.sqrt(np.float32(C))
    wqt = np.ascontiguousarray(Wq.astype(np.float32).T * sc)
    bqv = np.ascontiguousarray((bq.astype(np.float32) * sc).reshape(C, 1))
    wkt = np.ascontiguousarray(Wk.astype(np.float32).T)
    bkv = np.ascontiguousarray(bk.astype(np.float32).reshape(C, 1))

    ident = np.eye(C, dtype=np.float32)
    ones = np.ones((C, C), dtype=np.float32).astype(ml_dtypes.bfloat16)

    # 0/1 band masks on the two valid strips (same for both rows):
    # chunk1: key p vs query w in [0, SW);  chunk2: key 128+p vs query 120+j
    maskc = np.zeros((C, 2, SW), dtype=np.float32)
    for p in range(C):
        for w in range(SW):
            if abs(p - w) <= R:
                maskc[p, 0, w] = 1.0
            if abs((128 + p) - (W - SW + w)) <= R:
                maskc[p, 1, w] = 1.0
    maskc = np.broadcast_to(maskc[:, None], (C, 2, 2, SW))
    maskc = np.ascontiguousarray(maskc).astype(ml_dtypes.bfloat16)

    # oob count per query w (pre-divided by 128: the ones-matmul sums over
    # 128 partitions), same row repeated on all partitions, two rows
    wgrid = np.arange(W)
    oob_row = (np.maximum(0, R - wgrid) + np.maximum(0, wgrid - (W - 1 - R))) / 128.0
    oob_bc = np.tile(oob_row.astype(np.float32), (C, 2)).astype(ml_dtypes.bfloat16)
    return wqt, bqv, wkt, bkv, maskc, oob_bc, ident, ones


def core_inputs(feature, position, Wq, bq, Wk, bk):
    wqt, bqv, wkt, bkv, maskc, oob_bc, ident, ones = host_consts(Wq, bq, Wk, bk)
    in_maps = []
    for i in range(NCORES):
        b = i // CORES_PER_B
        h0 = (i % CORES_PER_B) * ROWS
        in_maps.append(
            {
                "feature": np.ascontiguousarray(
                    feature[b, :, h0 : h0 + ROWS, :], dtype=np.float32
                ),
                "position": np.ascontiguousarray(
                    position[b, :, h0 : h0 + ROWS, :], dtype=np.float32
                ),
                "wqt": wqt,
                "ident": ident,
                "ones": ones,
                "wkt": wkt,
                "bqv": bqv,
                "bkv": bkv,
                "maskc": maskc,
                "oob_bc": oob_bc,
            }
        )
    return in_maps


def kernel(feature, position, Wq, bq, Wk, bk):
    feature = np.asarray(feature, dtype=np.float32)
    position = np.asarray(position, dtype=np.float32)
    Wq = np.asarray(Wq, dtype=np.float32)
    bq = np.asarray(bq, dtype=np.float32)
    Wk = np.asarray(Wk, dtype=np.float32)
    bk = np.asarray(bk, dtype=np.float32)
    in_maps = core_inputs(feature, position, Wq, bq, Wk, bk)
    nc = build_nc()
    res = run_bass_kernel_spmd(nc, in_maps, list(range(NCORES)))
    out = np.empty((B, C, H, W), dtype=np.float32)
    for i in range(NCORES):
        b = i // CORES_PER_B
        h0 = (i % CORES_PER_B) * ROWS
        out[b, :, h0 : h0 + ROWS, :] = res.results[i]["out"]
    return out
